# revision 1
# baseline (speedup 1.0000x reference)
"""NonLocalAttention Trainium2 kernel.

Reference computation (N=2, C=64, CR=32, H=W=96, HW=9216):
    e1  = PReLU(w1 @ inputa + b1)   # [N,32,HW]   (queries)
    e2  = PReLU(w2 @ inputb + b2)   # [N,32,HW]   (keys)
    asm = PReLU(wa @ inputa + ba)   # [N,64,HW]   (values)
    out = softmax(e1^T e2, axis=keys) @ asm^T + inputa

Sharding: 8 cores = 2 batches x 4 query-chunks of 2304 rows. Each core gets
its batch's full inputa/inputb (for keys/values) plus its query chunk, and
writes a disjoint [64, 2304] slice of the output. No collectives.

Per-core kernel (flash-style, never materializes [HW,HW]):
  - conv biases are folded into the matmuls by augmenting the contraction
    dim with a ones-row; inputs are host-padded to 128 contraction rows
    (row 64 = ones, rows 65.. = zeros) because matmuls whose inputs span
    fewer than 128 partitions stream at HALF rate on this silicon
    (measured: 453 ns vs 246 ns per 512-row bf16 matmul).
  - e1/e2 are likewise stored in [128, *] tiles with rows 32..127 zeroed
    so the QK matmul contracts K=128 (zeros contribute nothing).
  - PReLU slope is exactly 0.25 (power of two), so prelu(x) == max(x, .25x)
    exactly; two DVE ops (walrus allows one PSUM operand per op).
  - attention uses the S^T = e2^T e1 orientation: keys land on the PSUM
    partition dim, so the PV matmul needs no transposes at all, and an
    all-ones 65th column in the value tiles makes the PV matmul emit the
    softmax denominator as PSUM row 64 for free.
  - scores are bounded (|s| <= 32 * max|e1| * max|e2| << 88) so exp needs
    no max-subtraction; softmax normalization divides at the end in fp32,
    with the reciprocal row broadcast across partitions by a 0-stride DMA.
  - everything on the PE is bf16 (fp32 runs a 4-pass mode at 1/4 rate;
    float32r is not plumbed to the fast replicated path and measures both
    slow AND lossy). PSUM accumulation stays fp32; bf16 rounding errors
    average out across the 9216-key softmax sum.
"""

import numpy as np

C = 64
CR = 32
KP = 128  # padded contraction size (ones row at 64, zeros above)
HW = 9216
QCH = 2304  # query rows per core
NKT = HW // 128  # 72 key tiles
NCORES = 8
QBLOCKS = [(0, 512), (512, 512), (1024, 512), (1536, 512), (2048, 256)]


def _ensure_ntff_hook():
    """Best-effort registration of the axon NTFF profile hook; the agent
    image's antenv package lacks axon_hooks, which would make any traced
    run crash on import instead of degrading."""
    import sys
    import types

    try:
        import antenv.axon_hooks  # noqa: F401

        return
    except ImportError:
        pass
    try:
        import antenv
        from trn_agent_boot.trn_boot import _ntff_profile_via_ctypes

        hook = _ntff_profile_via_ctypes("/opt/axon/libaxon_pjrt.so")
        mod = types.ModuleType("antenv.axon_hooks")
        _h = [hook]
        mod.get_axon_ntff_profile_hook = lambda: _h[0]
        mod.set_axon_ntff_profile_hook = lambda h: _h.__setitem__(0, h)
        sys.modules["antenv.axon_hooks"] = mod
        antenv.axon_hooks = mod
    except Exception:
        pass


def build_program(a1: float, a2: float, aa: float):
    import concourse.bacc as bacc
    import concourse.tile as tile
    from concourse import mybir

    f32 = mybir.dt.float32
    bf16 = mybir.dt.bfloat16
    AF = mybir.ActivationFunctionType

    nc = bacc.Bacc()
    xa = nc.dram_tensor("xa", [KP, HW], bf16, kind="ExternalInput")
    xb = nc.dram_tensor("xb", [KP, HW], bf16, kind="ExternalInput")
    xq = nc.dram_tensor("xq", [KP, QCH], bf16, kind="ExternalInput")
    xqf = nc.dram_tensor("xqf", [C, QCH], f32, kind="ExternalInput")
    w1t = nc.dram_tensor("w1t", [KP, CR], bf16, kind="ExternalInput")
    w2t = nc.dram_tensor("w2t", [KP, CR], bf16, kind="ExternalInput")
    wat = nc.dram_tensor("wat", [KP, C], bf16, kind="ExternalInput")
    out = nc.dram_tensor("out", [C, QCH], f32, kind="ExternalOutput")

    with tile.TileContext(nc) as tc:
        with (
            tc.tile_pool(name="consts", bufs=1) as consts,
            tc.tile_pool(name="big", bufs=1) as big,
            tc.tile_pool(name="ps", bufs=2, space="PSUM") as ps,
            tc.tile_pool(name="po", bufs=1, space="PSUM") as ps_o,
            tc.tile_pool(name="pt", bufs=3) as ptile,
            tc.tile_pool(name="work", bufs=2) as work,
        ):
            # --- constants / weights -------------------------------------
            w1_sb = consts.tile([KP, CR], bf16, tag="w1")
            nc.sync.dma_start(w1_sb[:], w1t[:])
            w2_sb = consts.tile([KP, CR], bf16, tag="w2")
            nc.sync.dma_start(w2_sb[:], w2t[:])
            wa_sb = consts.tile([KP, C], bf16, tag="wa")
            nc.sync.dma_start(wa_sb[:], wat[:])

            # --- activations in, chunked for DMA/compute overlap ---------
            xa_sb = big.tile([KP, HW], bf16, tag="xa")
            xb_sb = big.tile([KP, HW], bf16, tag="xb")
            xq_sb = big.tile([KP, QCH], bf16, tag="xq")
            xqf_sb = big.tile([C, QCH], f32, tag="xqf")
            for off in range(0, HW, QCH):
                nc.sync.dma_start(xa_sb[:, off : off + QCH], xa[:, off : off + QCH])
                nc.sync.dma_start(xb_sb[:, off : off + QCH], xb[:, off : off + QCH])
            nc.sync.dma_start(xq_sb[:], xq[:])
            nc.sync.dma_start(xqf_sb[:], xqf[:])

            # --- e1 = prelu(w1 @ xq + b1): rows 0:32 of [128, QCH] -------
            # rows 32:128 zeroed so QK can contract K=128 at full rate.
            e1_sb = big.tile([KP, QCH], bf16, tag="e1")
            for p0 in range(CR, KP, 32):  # start partitions limited to +32 spans
                nc.gpsimd.memset(e1_sb[p0 : p0 + 32, :], 0.0)
            for off, nq in QBLOCKS:
                pse = ps.tile([CR, nq], f32, tag="ps")
                nc.tensor.matmul(
                    pse[:], w1_sb[:], xq_sb[:, off : off + nq],
                    start=True, stop=True,
                )
                ya = work.tile([CR, nq], f32, tag="ya1")
                nc.vector.tensor_scalar_mul(ya[:], pse[:], a1)
                nc.vector.tensor_max(e1_sb[0:CR, off : off + nq], ya[:], pse[:])

            # --- e2 = prelu(w2 @ xb + b2): rows 0:32 of [128, HW] --------
            e2_sb = big.tile([KP, HW], bf16, tag="e2")
            for p0 in range(CR, KP, 32):
                nc.gpsimd.memset(e2_sb[p0 : p0 + 32, :], 0.0)
            for off in range(0, HW, 512):
                pse = ps.tile([CR, 512], f32, tag="ps")
                nc.tensor.matmul(
                    pse[:], w2_sb[:], xb_sb[:, off : off + 512],
                    start=True, stop=True,
                )
                ya = work.tile([CR, 512], f32, tag="ya2")
                nc.vector.tensor_scalar_mul(ya[:], pse[:], a2)
                nc.vector.tensor_max(e2_sb[0:CR, off : off + 512], ya[:], pse[:])

            # --- v_aug tiles: [128, 65] bf16 per key tile, col 64 = ones -
            # v = asm^T computed directly transposed: per key tile i,
            # psum[128,64] = xa[:, i*128:(i+1)*128]^T @ wat.
            v_all = big.tile([128, NKT * 65], bf16, tag="vall")
            v3 = v_all[:].rearrange("p (t c) -> p t c", c=65)
            nc.gpsimd.memset(v3[:, :, 64:65], 1.0)
            for grp in range(NKT // 8):  # 8 key tiles per psum bank batch
                psv = ps.tile([128, 512], f32, tag="ps")
                for j in range(8):
                    i = grp * 8 + j
                    nc.tensor.matmul(
                        psv[:, j * 64 : (j + 1) * 64],
                        xa_sb[:, i * 128 : (i + 1) * 128],
                        wa_sb[:],
                        start=(j == 0), stop=(j == 7),
                    )
                psv3 = psv[:].rearrange("p (t c) -> p t c", c=64)
                yv = work.tile([128, 512], f32, tag="yv")
                yv3 = yv[:].rearrange("p (t c) -> p t c", c=64)
                nc.vector.tensor_scalar_mul(yv[:], psv[:], aa)
                nc.vector.tensor_max(
                    v3[:, grp * 8 : (grp + 1) * 8, 0:64], yv3[:], psv3[:]
                )

            # --- attention: per q-block, loop key tiles ------------------
            # S^T psum batches 3 key tiles (3 banks) per exp op.
            for off, nq in QBLOCKS:
                kt_per_ps = 1536 // nq  # 3 at nq=512, 6 at nq=256
                po = ps_o.tile([C + 1, nq], f32, tag="po")
                for g in range(NKT // kt_per_ps):
                    pss = ps.tile([128, 1536], f32, tag="ps")
                    for j in range(kt_per_ps):
                        i = g * kt_per_ps + j
                        colb = j * nq * 4  # byte offset of this matmul
                        nc.tensor.matmul(
                            pss[:, j * nq : (j + 1) * nq],
                            e2_sb[:, i * 128 : (i + 1) * 128],
                            e1_sb[:, off : off + nq],
                            start=(colb % 2048 == 0),
                            stop=((colb + nq * 4) % 2048 == 0),
                        )
                    pt = ptile.tile([128, 1536], bf16, tag="pt")
                    nc.scalar.activation(pt[:], pss[:], AF.Exp)
                    for j in range(kt_per_ps):
                        i = g * kt_per_ps + j
                        nc.tensor.matmul(
                            po[:],
                            v_all[:, i * 65 : (i + 1) * 65],
                            pt[:, j * nq : (j + 1) * nq],
                            start=(i == 0), stop=(i == NKT - 1),
                        )
                # epilogue: out = po[0:64] / po[64] + xq   (all fp32)
                rec = work.tile([1, nq], f32, tag="rec")
                nc.vector.reciprocal(rec[:], po[C : C + 1, :])
                rb = work.tile([C, nq], f32, tag="rb")
                rec_rep = rec[0:1, :].rearrange("a (b c) -> a b c", b=1)
                nc.sync.dma_start(rb[:], rec_rep.to_broadcast((1, C, nq)))
                osb = work.tile([C, nq], f32, tag="osb")
                nc.vector.tensor_mul(osb[:], rb[:], po[0:C, :])
                nc.vector.tensor_add(osb[:], osb[:], xqf_sb[:, off : off + nq])
                nc.sync.dma_start(out[:, off : off + nq], osb[:])
    nc.finalize()
    return nc


def run(inputs: dict, trace: bool = False, tmpdir: str | None = None):
    """Build, compile and run on 8 cores; returns (output, BassKernelResults)."""
    _ensure_ntff_hook()
    from concourse.bass_utils import run_bass_kernel_spmd

    inputa = np.asarray(inputs["inputa"], dtype=np.float32)
    inputb = np.asarray(inputs["inputb"], dtype=np.float32)
    w1 = np.asarray(inputs["w1"], dtype=np.float32)
    b1 = np.asarray(inputs["b1"], dtype=np.float32)
    w2 = np.asarray(inputs["w2"], dtype=np.float32)
    b2 = np.asarray(inputs["b2"], dtype=np.float32)
    wa = np.asarray(inputs["wa"], dtype=np.float32)
    ba = np.asarray(inputs["ba"], dtype=np.float32)
    a1 = float(np.asarray(inputs["a1"]).reshape(-1)[0])
    a2 = float(np.asarray(inputs["a2"]).reshape(-1)[0])
    aa = float(np.asarray(inputs["aa"]).reshape(-1)[0])

    N, Cc, H, W = inputa.shape
    assert (N, Cc, H * W) == (2, C, HW), inputa.shape
    chunks_per_batch = NCORES // N  # 4

    import ml_dtypes

    bf = ml_dtypes.bfloat16

    def pad128(m):
        """[rows, n] -> [128, n] with a ones row at 64 and zeros above."""
        rows, n = m.shape
        out_ = np.zeros((KP, n), np.float32)
        out_[:rows] = m
        out_[C] = 1.0 if rows == C else out_[C]
        return out_

    xa_n = inputa.reshape(N, C, HW)
    xb_n = inputb.reshape(N, C, HW)

    def aug128(x):
        p = np.zeros((KP, x.shape[1]), np.float32)
        p[:C] = x
        p[C] = 1.0
        return p.astype(bf)

    def wpad(wt, b):
        p = np.zeros((KP, wt.shape[1]), np.float32)
        p[:C] = wt
        p[C] = b
        return p.astype(bf)

    w1t_aug = wpad(w1.T, b1)
    w2t_aug = wpad(w2.T, b2)
    wat_aug = wpad(wa.T, ba)

    in_maps = []
    for core in range(NCORES):
        b, chunk = divmod(core, chunks_per_batch)
        xa_aug = aug128(xa_n[b])
        xb_aug = aug128(xb_n[b])
        xq_aug = np.ascontiguousarray(
            xa_aug[:, chunk * QCH : (chunk + 1) * QCH]
        )
        xqf = np.ascontiguousarray(
            xa_n[b][:, chunk * QCH : (chunk + 1) * QCH]
        )
        in_maps.append(
            {
                "xa": xa_aug,
                "xb": xb_aug,
                "xq": xq_aug,
                "xqf": xqf,
                "w1t": w1t_aug,
                "w2t": w2t_aug,
                "wat": wat_aug,
            }
        )

    nc = build_program(a1, a2, aa)
    res = run_bass_kernel_spmd(
        nc, in_maps, list(range(NCORES)), trace=trace, tmpdir=tmpdir
    )

    out = np.empty((N, C, HW), np.float32)
    for core in range(NCORES):
        b, chunk = divmod(core, chunks_per_batch)
        out[b, :, chunk * QCH : (chunk + 1) * QCH] = res.results[core]["out"]
    return out.reshape(N, C, H, W), res


def kernel(**inputs) -> np.ndarray:
    out, _ = run(inputs, trace=False)
    return out



# revision 9
# speedup vs baseline: 1.3024x; 1.3024x over previous
"""NonLocalAttention Trainium2 kernel (v2 — row-tiled QK, pipelined softmax).

Reference computation (N=2, C=64, CR=32, H=W=96, HW=9216):
    e1  = PReLU(w1 @ inputa + b1)   # [N,32,HW]   (queries)
    e2  = PReLU(w2 @ inputb + b2)   # [N,32,HW]   (keys)
    asm = PReLU(wa @ inputa + ba)   # [N,64,HW]   (values)
    out = softmax(e1^T e2, axis=keys) @ asm^T + inputa

Sharding: 8 cores = 2 batches x 4 query-chunks of 2304 rows. Softmax is
key-order invariant, so the host ROTATES the key/value columns per core so
that each core's query chunk is always columns 0:2304 — one SPMD program,
no per-core offsets, and no separate xq input. No collectives.

Per-core kernel (flash-style, never materializes [HW,HW]):
  - QK is ROW-TILED: the contraction is only CR=32, so three K=32 matmuls
    run CONCURRENTLY in PE row-groups 0..2 (tile_position via base_partition),
    each producing S^T for one 128-key tile. ~2.4x faster than one K=128
    matmul and needs no zero-padding of e1/e2.
  - e1 is computed with a column-replicated stationary w1r [128,128] (4
    copies of w1^T+bias) so the PReLU output lands replicated in all four
    32-partition groups, ready to be the row-tiled QK moving operand.
  - e2 is computed with COL-TILED matmuls (stationary w2 at col-group j)
    so key tile 3g+j lands directly at partitions 32j, col block g.
  - conv biases fold into the matmuls via an augmented ones-row (row 64);
    moving operands are [65,HW] from the host plus one DVE memset of rows
    65:128 (keeps K=128 full-rate contraction for the convs).
  - attention loop is software-pipelined: QK(g+1) is emitted BEFORE PV(g)
    so the tensor engine never waits on exp(g); the PV accumulator po is
    double-buffered so the softmax-divide epilogue of block b overlaps
    block b+1 (the baseline stalled 6-9us per block here, which also
    re-throttled the PE clock via the HAM activity monitor).
  - an all-ones 65th column in the value tiles makes the PV matmul emit the
    softmax denominator as PSUM row 64 for free; the epilogue uses the fast
    approximate reciprocal (~18 bits, plenty vs the 2e-2 gate).
  - everything on the PE is bf16; PSUM stays fp32.
"""

import numpy as np

C = 64
CR = 32
HW = 9216
QCH = 2304  # query rows per core
NKT = HW // 128  # 72 key tiles
R = 3  # row-tiled QK tiles per group
NG = NKT // R  # 24 key groups
NCORES = 8
QBLOCKS = [(0, 512), (512, 512), (1024, 512), (1536, 512), (2048, 256)]
PIPELINE = True  # emit QK(g+1) before PV(g)


def _ensure_ntff_hook():
    """Best-effort registration of the axon NTFF profile hook; the agent
    image's antenv package lacks axon_hooks, which would make any traced
    run crash on import instead of degrading."""
    import sys
    import types

    try:
        import antenv.axon_hooks  # noqa: F401

        return
    except ImportError:
        pass
    try:
        import antenv
        from trn_agent_boot.trn_boot import _ntff_profile_via_ctypes

        hook = _ntff_profile_via_ctypes("/opt/axon/libaxon_pjrt.so")
        mod = types.ModuleType("antenv.axon_hooks")
        _h = [hook]
        mod.get_axon_ntff_profile_hook = lambda: _h[0]
        mod.set_axon_ntff_profile_hook = lambda h: _h.__setitem__(0, h)
        sys.modules["antenv.axon_hooks"] = mod
        antenv.axon_hooks = mod
    except Exception:
        pass


def build_program(a1: float, a2: float, aa: float):
    import concourse.bacc as bacc
    import concourse.tile as tile
    from concourse import mybir

    f32 = mybir.dt.float32
    bf16 = mybir.dt.bfloat16
    AF = mybir.ActivationFunctionType

    nc = bacc.Bacc()
    xa = nc.dram_tensor("xa", [C + 1, HW], bf16, kind="ExternalInput")
    xb = nc.dram_tensor("xb", [C + 1, HW], bf16, kind="ExternalInput")
    xqf = nc.dram_tensor("xqf", [C, QCH], f32, kind="ExternalInput")
    w1r = nc.dram_tensor("w1r", [128, 128], bf16, kind="ExternalInput")
    w2a = nc.dram_tensor("w2a", [128, CR], bf16, kind="ExternalInput")
    waa = nc.dram_tensor("waa", [128, C], bf16, kind="ExternalInput")
    out = nc.dram_tensor("out", [C, QCH], f32, kind="ExternalOutput")

    with tile.TileContext(nc) as tc:
        with (
            tc.tile_pool(name="consts", bufs=1) as consts,
            tc.tile_pool(name="big", bufs=1) as big,
            tc.tile_pool(name="ps", bufs=2, space="PSUM") as ps,
            tc.tile_pool(name="po", bufs=2, space="PSUM") as ps_o,
            tc.tile_pool(name="pt", bufs=3) as ptile,
            tc.tile_pool(name="work", bufs=2) as work,
        ):
            # --- constants / weights -------------------------------------
            w1r_sb = consts.tile([128, 128], bf16, tag="w1r")
            nc.sync.dma_start(w1r_sb[:], w1r[:])
            w2a_sb = consts.tile([128, CR], bf16, tag="w2a")
            nc.sync.dma_start(w2a_sb[:], w2a[:])
            waa_sb = consts.tile([128, C], bf16, tag="waa")
            nc.sync.dma_start(waa_sb[:], waa[:])

            # --- activations in ------------------------------------------
            xa_sb = big.tile([128, HW], bf16, tag="xa")
            xb_sb = big.tile([128, HW], bf16, tag="xb")
            xqf_sb = big.tile([C, QCH], f32, tag="xqf")
            # zero the pad rows first (32-aligned partition start required);
            # the DMAs below then overwrite row 64 with the ones row
            nc.vector.memset(xa_sb[C:128, :], 0.0)
            nc.vector.memset(xb_sb[C:128, :], 0.0)
            # query chunk of xa first (gates e1), then xb (gates e2)
            nc.sync.dma_start(xa_sb[0 : C + 1, 0:QCH], xa[:, 0:QCH])
            for off in range(0, HW, QCH):
                nc.sync.dma_start(xb_sb[0 : C + 1, off : off + QCH], xb[:, off : off + QCH])
            for off in range(QCH, HW, QCH):
                nc.sync.dma_start(xa_sb[0 : C + 1, off : off + QCH], xa[:, off : off + QCH])
            nc.sync.dma_start(xqf_sb[:], xqf[:])

            e1_sb = big.tile([128, QCH], bf16, tag="e1")
            e2_sb = big.tile([96, NG * 128], bf16, tag="e2")
            v_all = big.tile([128, NKT * 65], bf16, tag="vall")
            v3 = v_all[:].rearrange("p (t c) -> p t c", c=65)
            nc.vector.memset(v3[:, :, 64:65], 1.0)

            # --- e1 = prelu(w1 @ xq + b1), replicated in 4 row groups ----
            for off, nq in QBLOCKS:
                pse = ps.tile([128, 1536], f32, tag="ps")
                nc.tensor.matmul(
                    pse[:, 0:nq], w1r_sb[:], xa_sb[:, off : off + nq],
                    start=True, stop=True,
                )
                ya = work.tile([128, 512], f32, tag="ya1")
                nc.vector.tensor_scalar_mul(ya[:, 0:nq], pse[:, 0:nq], a1)
                nc.vector.tensor_max(e1_sb[:, off : off + nq], ya[:, 0:nq], pse[:, 0:nq])

            # --- e2 = prelu(w2 @ xb + b2): key tile 3g+j at partitions 32j,
            # col block g of e2_sb, via col-tiled matmuls ------------------
            for t in range(2):  # two [128,1536] psum batches of 12 col blocks
                pse = ps.tile([128, 1536], f32, tag="ps")
                for m in range(12):
                    g = 12 * t + m
                    for j in range(R):
                        kt = R * g + j
                        nc.tensor.matmul(
                            pse[32 * j : 32 * (j + 1), m * 128 : (m + 1) * 128],
                            w2a_sb[:],
                            xb_sb[:, kt * 128 : (kt + 1) * 128],
                            start=True, stop=True,
                        )
                ya = work.tile([96, 1536], f32, tag="ya2")
                nc.vector.tensor_scalar_mul(ya[:], pse[0:96, :], a2)
                nc.vector.tensor_max(
                    e2_sb[:, t * 1536 : (t + 1) * 1536], ya[:], pse[0:96, :]
                )

            # --- v_aug tiles: [128, 65] bf16 per key tile, col 64 = ones -
            # per key tile i, psum[128,64] = xa[:, i*128:(i+1)*128]^T @ waa.
            for grp in range(NKT // 24):  # 24 key tiles per [128,1536] batch
                psv = ps.tile([128, 1536], f32, tag="ps")
                for j in range(24):
                    i = grp * 24 + j
                    nc.tensor.matmul(
                        psv[:, j * 64 : (j + 1) * 64],
                        xa_sb[:, i * 128 : (i + 1) * 128],
                        waa_sb[:],
                        start=(j % 8 == 0), stop=(j % 8 == 7),
                    )
                psv3 = psv[:].rearrange("p (t c) -> p t c", c=64)
                yv = work.tile([128, 1536], f32, tag="yv")
                yv3 = yv[:].rearrange("p (t c) -> p t c", c=64)
                nc.vector.tensor_scalar_mul(yv[:], psv[:], aa)
                nc.vector.tensor_max(
                    v3[:, grp * 24 : (grp + 1) * 24, 0:64], yv3[:], psv3[:]
                )

            # --- attention: per q-block, software-pipelined over key groups
            for off, nq in QBLOCKS:
                po = ps_o.tile([C + 1, 512], f32, tag="po")
                pt_prev = None
                g_prev = -1
                for g in range(NG):
                    # QK: 3 concurrent row-tiled K=32 matmuls, one PSUM bank
                    # each (tile j at column j*512 even when nq=256, so no
                    # start/stop group ever spans tile positions)
                    pss = ps.tile([128, 1536], f32, tag="ps")
                    for j in range(R):
                        nc.tensor.matmul(
                            pss[:, j * 512 : j * 512 + nq],
                            e2_sb[32 * j : 32 * (j + 1), g * 128 : (g + 1) * 128],
                            e1_sb[32 * j : 32 * (j + 1), off : off + nq],
                            start=True, stop=True,
                        )
                    pt = ptile.tile([128, 1536], bf16, tag="pt")
                    if nq == 512:
                        nc.scalar.activation(pt[:], pss[:], AF.Exp)
                    else:
                        pss3 = pss[:].rearrange("p (t c) -> p t c", c=512)
                        pt3 = pt[:].rearrange("p (t c) -> p t c", c=512)
                        nc.scalar.activation(
                            pt3[:, :, 0:nq], pss3[:, :, 0:nq], AF.Exp
                        )

                    def emit_pv(ptx, gx):
                        for j in range(R):
                            kt = R * gx + j
                            nc.tensor.matmul(
                                po[:, 0:nq],
                                v_all[:, kt * 65 : (kt + 1) * 65],
                                ptx[:, j * 512 : j * 512 + nq],
                                start=(kt == 0), stop=(kt == NKT - 1),
                            )

                    if PIPELINE:
                        # PV of the PREVIOUS group — emitted after QK(g) so
                        # the tensor queue never blocks on exp(g)
                        if pt_prev is not None:
                            emit_pv(pt_prev, g_prev)
                        pt_prev = pt
                        g_prev = g
                    else:
                        emit_pv(pt, g)
                if PIPELINE:
                    emit_pv(pt_prev, g_prev)
                # epilogue: out = po[0:64] / po[64] + xq   (all fp32)
                rec = work.tile([1, 512], f32, tag="rec")
                nc.vector.reciprocal(rec[0:1, 0:nq], po[C : C + 1, 0:nq])
                rb = work.tile([C, 512], f32, tag="rb")
                rec_rep = rec[0:1, 0:nq].rearrange("a (b c) -> a b c", b=1)
                nc.sync.dma_start(rb[:, 0:nq], rec_rep.to_broadcast((1, C, nq)))
                osb = work.tile([C, 512], f32, tag="osb")
                nc.vector.tensor_mul(osb[:, 0:nq], rb[:, 0:nq], po[0:C, 0:nq])
                nc.vector.tensor_add(osb[:, 0:nq], osb[:, 0:nq], xqf_sb[:, off : off + nq])
                nc.sync.dma_start(out[:, off : off + nq], osb[:, 0:nq])
    nc.finalize()
    return nc


def run(inputs: dict, trace: bool = False, tmpdir: str | None = None):
    """Build, compile and run on 8 cores; returns (output, BassKernelResults)."""
    _ensure_ntff_hook()
    from concourse.bass_utils import run_bass_kernel_spmd

    inputa = np.asarray(inputs["inputa"], dtype=np.float32)
    inputb = np.asarray(inputs["inputb"], dtype=np.float32)
    w1 = np.asarray(inputs["w1"], dtype=np.float32)
    b1 = np.asarray(inputs["b1"], dtype=np.float32)
    w2 = np.asarray(inputs["w2"], dtype=np.float32)
    b2 = np.asarray(inputs["b2"], dtype=np.float32)
    wa = np.asarray(inputs["wa"], dtype=np.float32)
    ba = np.asarray(inputs["ba"], dtype=np.float32)
    a1 = float(np.asarray(inputs["a1"]).reshape(-1)[0])
    a2 = float(np.asarray(inputs["a2"]).reshape(-1)[0])
    aa = float(np.asarray(inputs["aa"]).reshape(-1)[0])

    N, Cc, H, W = inputa.shape
    assert (N, Cc, H * W) == (2, C, HW), inputa.shape
    chunks_per_batch = NCORES // N  # 4

    import ml_dtypes

    bf = ml_dtypes.bfloat16

    xa_n = inputa.reshape(N, C, HW)
    xb_n = inputb.reshape(N, C, HW)

    def aug65(x):
        """[64, HW] -> [65, HW] bf16 with a ones row at 64."""
        p = np.empty((C + 1, x.shape[1]), np.float32)
        p[:C] = x
        p[C] = 1.0
        return p.astype(bf)

    def wpad(wt, b, rep=1):
        """[64, M] weights^T + bias row at 64, zeros to 128 rows; optional
        column replication for the row-tiled QK moving-operand layout."""
        m = wt.shape[1]
        p = np.zeros((128, m * rep), np.float32)
        for r in range(rep):
            p[:C, r * m : (r + 1) * m] = wt
            p[C, r * m : (r + 1) * m] = b
        return p.astype(bf)

    w1r_aug = wpad(w1.T, b1, rep=4)  # [128, 128]
    w2a_aug = wpad(w2.T, b2)  # [128, 32]
    waa_aug = wpad(wa.T, ba)  # [128, 64]

    in_maps = []
    for core in range(NCORES):
        b, chunk = divmod(core, chunks_per_batch)
        qoff = chunk * QCH
        # rotate keys/values so this core's queries are columns 0:QCH
        # (softmax over keys is invariant to the key order)
        rot = np.concatenate([xa_n[b][:, qoff:], xa_n[b][:, :qoff]], axis=1)
        rot_b = np.concatenate([xb_n[b][:, qoff:], xb_n[b][:, :qoff]], axis=1)
        in_maps.append(
            {
                "xa": aug65(rot),
                "xb": aug65(rot_b),
                "xqf": np.ascontiguousarray(rot[:, 0:QCH]),
                "w1r": w1r_aug,
                "w2a": w2a_aug,
                "waa": waa_aug,
            }
        )

    nc = build_program(a1, a2, aa)
    res = run_bass_kernel_spmd(
        nc, in_maps, list(range(NCORES)), trace=trace, tmpdir=tmpdir
    )

    out = np.empty((N, C, HW), np.float32)
    for core in range(NCORES):
        b, chunk = divmod(core, chunks_per_batch)
        out[b, :, chunk * QCH : (chunk + 1) * QCH] = res.results[core]["out"]
    return out.reshape(N, C, H, W), res


def kernel(**inputs) -> np.ndarray:
    out, _ = run(inputs, trace=False)
    return out
